# revision 48
# baseline (speedup 1.0000x reference)
"""Trainium2 Bass kernel for nn_DLKAConvBlock (B=4, C=64, H=W=256) on 8 NeuronCores.

Sharding: data-parallel over (batch, H-half): core = 2*b + half, each core
computes output rows [r0, r0+128) of image b (r0 = 128*half), working in a
local row coordinate frame l (img row = r0 + l) so the SPMD program is
identical across cores; all per-core differences are carried by input data
(host-shifted, zero-padded slices + row-validity masks).

Pipeline per core (all on-device):
  conv3x3 full image (stats only) -> instance-norm scale/bias
  conv3x3 on local rows -> h_local
  t = gelu(p1 @ norm(h)) (masked to valid rows) -> t_nchw + t_pad (NHWC, padded)
  off0 = conv5x5(t); deformable-depthwise-5x5 via SWDGE dma_gather of
  2x1-pixel pairs (2 rows per tap) + DVE bilinear combine + PE
  transpose-accumulate over taps -> a1 (rows [-16,144))
  offs = conv7x7-dil3(a1); deformable-depthwise-7x7 similarly -> a2
  tail: g1, u*a, p2, +shortcut, leaky-relu -> out rows [0,128)
"""
import os
import sys
from contextlib import ExitStack

import numpy as np

for _p in ("/opt/trn_rl_repo", "/root/.axon_site/_ro/trn_rl_repo"):
    if os.path.isdir(_p) and _p not in sys.path:
        sys.path.insert(0, _p)

import concourse.bass as bass
import concourse.bacc as bacc
import concourse.mybir as mybir
from concourse import tile
from concourse.bass_utils import run_bass_kernel_spmd

F32 = mybir.dt.float32
F16 = mybir.dt.float16
BF16 = mybir.dt.bfloat16
F8 = mybir.dt.float8e4
I16 = mybir.dt.int16
ALU = mybir.AluOpType
ACTF = mybir.ActivationFunctionType
F32R = mybir.dt.float32r

B, C, H, W = 4, 64, 256, 256
EPS = 1e-5
N_CORES = 8

# local-frame regions
HL0, HL1 = -24, 152          # h_local / t rows
NHROWS = HL1 - HL0           # 176
A1L0, A1L1 = -16, 144        # a1 rows
NA1ROWS = A1L1 - A1L0        # 160
PADR = 32                    # nhwc row pad (array row = l + 32)
PADC = 16                    # nhwc col pad
NPR = 192                    # nhwc rows: l in [-32, 160)
NPC = 288                    # nhwc cols: x in [-16, 272)
K1, K2 = 25, 49

D1_CHUNKS = [(-16, 48), (48, 112), (112, 144)]
D2_CHUNKS = [(0, 64), (64, 128)]
WIN1 = 8    # deform1 window margin rows (reach = 2 + |off|<=4 + 1)
WIN2 = 16   # deform2 window margin rows (reach = 9 + |off|<=4 + 1)


def _ap_raw(t_handle, offset, pattern):
    """Build an AP with an explicit [step, count] pattern on a tensor handle."""
    return bass.AP(t_handle, offset, [list(p) for p in pattern])


def build_program():
    nc = bacc.Bacc("TRN2", target_bir_lowering=False, debug=False,
                   enable_asserts=False, num_swdge_queues=4)

    # ---------------- external inputs ----------------
    # the other half's rows (not stats-covered by phase 2), padded by one
    # row each side: 128 rows -> 130 input rows; for instance-norm stats.
    # conv inputs carry a channel-duplicated column-shifted copy in the upper
    # 64 partitions (shift = dilation), so one matmul contracts TWO kernel
    # taps (128 partitions); odd taps get zero weights in the upper half.
    x_extra = nc.declare_dram_parameter("x_extra", [2 * C, 130, W + 2], F32R, isOutput=False)
    x_local = nc.declare_dram_parameter("x_local", [2 * C, NHROWS + 2, W + 2], F32R, isOutput=False)
    hmask = nc.declare_dram_parameter("hmask", [NHROWS // 2, C, 2], F32, isOutput=False)
    cw3 = nc.declare_dram_parameter("cw3", [2 * C, 6, C], F32R, isOutput=False)
    p1w = nc.declare_dram_parameter("p1w", [C, C], F32, isOutput=False)
    p1b = nc.declare_dram_parameter("p1b", [C, 1], F32, isOutput=False)
    off0w = nc.declare_dram_parameter("off0w", [2 * C, 15, 2 * K1], BF16, isOutput=False)
    off0b = nc.declare_dram_parameter("off0b", [2 * K1, 1], F32, isOutput=False)
    offsw = nc.declare_dram_parameter("offsw", [2 * C, 28, 2 * K2], BF16, isOutput=False)
    offsb = nc.declare_dram_parameter("offsb", [2 * K2, 1], F32, isOutput=False)
    dwk1 = nc.declare_dram_parameter("dwk1", [128, K1, C], F16, isOutput=False)
    dwk2 = nc.declare_dram_parameter("dwk2", [128, K2, C], F16, isOutput=False)
    g1w = nc.declare_dram_parameter("g1w", [C, C], BF16, isOutput=False)
    g1b = nc.declare_dram_parameter("g1b", [C, 1], F32, isOutput=False)
    p2w = nc.declare_dram_parameter("p2w", [C, C], BF16, isOutput=False)
    p2b = nc.declare_dram_parameter("p2b", [C, 1], F32, isOutput=False)
    identw = nc.declare_dram_parameter("identw", [128, 128], F32, isOutput=False)
    identw16 = nc.declare_dram_parameter("identw16", [128, 128], F16, isOutput=False)
    identw16b = nc.declare_dram_parameter("identw16b", [128, 128], BF16, isOutput=False)
    by1 = nc.declare_dram_parameter("by1", [128, 4 * K1], F32, isOutput=False)
    bx1 = nc.declare_dram_parameter("bx1", [128, 4 * K1], F32, isOutput=False)
    by2 = nc.declare_dram_parameter("by2", [128, 4 * K2], F32, isOutput=False)
    bx2 = nc.declare_dram_parameter("bx2", [128, 4 * K2], F32, isOutput=False)

    out_t = nc.declare_dram_parameter("out", [C, 128, W], F32, isOutput=True)

    # ---------------- internal DRAM ----------------
    # pads hold duplicated 2x2 corner blocks in fp8:
    # pad[y, x, :] = concat(v[y,x], v[y,x+1], v[y+1,x], v[y+1,x+1]),
    # so ONE 256B gather packet covers the full bilinear 2x2 patch.
    h_local = nc.dram_tensor("h_local", [C, NHROWS, W], F32)
    t_nchw = nc.dram_tensor("t_nchw", [2 * C, NHROWS, W + 4], BF16)
    t_pad = nc.dram_tensor("t_pad", [NPR, NPC, 4 * C], F8)
    a1_nchw = nc.dram_tensor("a1_nchw", [2 * C, NA1ROWS, W + 18], BF16)
    a1_pad = nc.dram_tensor("a1_pad", [NPR, NPC, 4 * C], F8)
    NSCR = 4
    scr1 = nc.dram_tensor("scr1", [NSCR, 128, 4 * K1], I16)
    scr2 = nc.dram_tensor("scr2", [NSCR, 128, 4 * K2], I16)

    PHASES = int(os.environ.get("KERNEL_PHASES", "5"))
    # dummy input whose shape encodes PHASES: busts the neuron NEFF cache,
    # whose key hashes only the HLO (BIR payload changes are invisible to it)
    nc.declare_dram_parameter("phtag", [1, PHASES + 1], F32, isOutput=False)

    with tile.TileContext(nc) as tc, ExitStack() as ctx:
        gather_regs = {n: nc.gpsimd.to_reg(n)
                       for n in (25 * 128, 24 * 128)}
        statics = ctx.enter_context(tc.tile_pool(name="statics", bufs=1))
        # resident static tiles
        s_cw3 = statics.tile([2 * C, 6, C], F32R)
        s_p1w = statics.tile([C, C], F32)
        s_p1b = statics.tile([C, 1], F32)
        s_g1w = statics.tile([C, C], BF16)
        s_g1b = statics.tile([C, 1], F32)
        s_p2w = statics.tile([C, C], BF16)
        s_p2b = statics.tile([C, 1], F32)
        s_id = statics.tile([128, 128], F32)
        s_id16 = statics.tile([128, 128], F16)
        s_id16b = statics.tile([128, 128], BF16)
        s_zero = statics.tile([128, 1024], F32)
        for dst, src in [(s_cw3, cw3), (s_p1w, p1w), (s_p1b, p1b),
                         (s_g1w, g1w), (s_g1b, g1b), (s_p2w, p2w), (s_p2b, p2b),
                         (s_id, identw), (s_id16, identw16), (s_id16b, identw16b)]:
            nc.sync.dma_start(dst[:], src[:])
        nc.vector.memset(s_zero[:], 0.0)

        # stats accumulators
        s_sum = statics.tile([C, 128], F32)
        s_sq = statics.tile([C, 128], F32)
        s_rstd = statics.tile([C, 1], F32)
        s_nbias = statics.tile([C, 1], F32)   # -mean*rstd
        s_cb = statics.tile([C, 1], F32)      # p2b + nbias
        s_tmp1 = statics.tile([C, 1], F32)
        s_tmp2 = statics.tile([C, 1], F32)
        nc.vector.memset(s_sum[:], 0.0)
        nc.vector.memset(s_sq[:], 0.0)

        # ---------------- memset DRAM pads ----------------
        for dram in (t_nchw, t_pad, a1_nchw, a1_pad):
            flat = dram[:].bitcast(F32).rearrange("a b c -> (a b c)")
            total = int(flat.shape[0])
            CH = 128 * 1024
            pos = 0
            while pos < total:
                n = min(CH, total - pos)
                rows = n // 1024
                if rows >= 1 and rows * 1024 == n:
                    nc.scalar.dma_start(
                        flat[pos:pos + n].rearrange("(p f) -> p f", p=rows),
                        s_zero[:rows, :])
                else:
                    nc.scalar.dma_start(flat[pos:pos + n], s_zero[0:1, :n])
                pos += n

        psum_conv = ctx.enter_context(
            tc.tile_pool(name="psum_conv", bufs=1, space="PSUM"))
        psum_tr = ctx.enter_context(
            tc.tile_pool(name="psum_tr", bufs=2, space="PSUM"))
        psum_acc = ctx.enter_context(
            tc.tile_pool(name="psum_acc", bufs=2, space="PSUM"))
        psum_tail = ctx.enter_context(
            tc.tile_pool(name="psum_tail", bufs=2, space="PSUM"))

        # ---------------- phase 1: conv3x3 stats on rows x_local misses ----
        P3 = [(0, 0), (0, 2), (1, 0), (1, 2), (2, 0), (2, 2)]
        with tc.tile_pool(name="ph1", bufs=3) as ph1:
          if PHASES >= 1:
            for g in range(64):
                xt = ph1.tile([2 * C, 4, W + 2], F32R, tag="xt")
                nc.sync.dma_start(xt[:], x_extra[:, 2 * g:2 * g + 4, :])
                ps = psum_conv.tile([C, 2 * W], F32, tag="conv")
                for j, (ky, kx) in enumerate(P3):
                    rhs = xt[:, ky:ky + 2, kx:kx + W]
                    nc.tensor.matmul(ps[:].rearrange("c (r w) -> c r w", r=2),
                                     s_cw3[:, j, :], rhs,
                                     start=(j == 0), stop=(j == 5))
                hd = ph1.tile([C, 2 * W], F32, tag="hd")
                nc.scalar.activation(hd[:], ps[:], ACTF.Copy,
                                     accum_out=s_sum[:, g:g + 1])
                sqd = ph1.tile([C, 2 * W], F32, tag="sqd")
                nc.scalar.activation(sqd[:], hd[:], ACTF.Square,
                                     accum_out=s_sq[:, g:g + 1])

        # ---------------- phase 2: h_local conv3x3 (+ own-row stats) -------
        with tc.tile_pool(name="ph2", bufs=3) as ph2:
          if PHASES >= 2:
            for g in range(NHROWS // 2):
                own = 12 <= g < 76  # local rows 2g-24, 2g-23 in [0, 128)
                xt = ph2.tile([2 * C, 4, W + 2], F32R, tag="xt")
                nc.sync.dma_start(xt[:], x_local[:, 2 * g:2 * g + 4, :])
                ps = psum_conv.tile([C, 2 * W], F32, tag="conv")
                for j, (ky, kx) in enumerate(P3):
                    nc.tensor.matmul(ps[:].rearrange("c (r w) -> c r w", r=2),
                                     s_cw3[:, j, :], xt[:, ky:ky + 2, kx:kx + W],
                                     start=(j == 0), stop=(j == 5))
                hsb = ph2.tile([C, 2 * W], F32, tag="hsb")
                if own:
                    nc.scalar.activation(hsb[:], ps[:], ACTF.Copy,
                                         accum_out=s_sum[:, 52 + g:53 + g])
                    sqd = ph2.tile([C, 2 * W], F32, tag="sqd")
                    nc.scalar.activation(sqd[:], hsb[:], ACTF.Square,
                                         accum_out=s_sq[:, 52 + g:53 + g])
                else:
                    nc.scalar.activation(hsb[:], ps[:], ACTF.Copy)
                nc.scalar.dma_start(h_local[:, 2 * g:2 * g + 2, :],
                                    hsb[:].rearrange("c (r w) -> c r w", r=2))

        # finalize stats
        nc.vector.tensor_reduce(s_tmp1[:], s_sum[:], mybir.AxisListType.X, ALU.add)
        nc.vector.tensor_reduce(s_tmp2[:], s_sq[:], mybir.AxisListType.X, ALU.add)
        inv_n = 1.0 / (H * W)
        # mean -> s_tmp1, E[x^2] -> s_tmp2
        nc.vector.tensor_scalar(s_tmp1[:], s_tmp1[:], inv_n, None, ALU.mult)
        nc.vector.tensor_scalar(s_tmp2[:], s_tmp2[:], inv_n, None, ALU.mult)
        # var = E[x^2] - mean^2 ; rstd = 1/sqrt(var+eps)
        var = statics.tile([C, 1], F32)
        # (mean*mean) - E[x^2] = -var ; then negate and add eps
        nc.vector.scalar_tensor_tensor(var[:], s_tmp1[:], s_tmp1[:], s_tmp2[:],
                                       ALU.mult, ALU.subtract)
        nc.vector.tensor_scalar(var[:], var[:], -1.0, EPS, ALU.mult, ALU.add)
        nc.scalar.sqrt(var[:], var[:])
        nc.vector.reciprocal(s_rstd[:], var[:])
        nc.vector.scalar_tensor_tensor(s_nbias[:], s_tmp1[:], -1.0, s_rstd[:],
                                       ALU.mult, ALU.mult)
        nc.vector.tensor_tensor(s_cb[:], s_p2b[:], s_nbias[:], ALU.add)

        # ---------------- phase 3: t = mask*gelu(p1 @ norm(h)) ----------------
        with tc.tile_pool(name="ph3", bufs=3) as ph3:
          if PHASES >= 3:
            for g in range(NHROWS // 2):
                hsb = ph3.tile([C, 2 * W], F32, tag="hld")
                nc.sync.dma_start(
                    hsb[:], h_local[:, 2 * g:2 * g + 2, :].rearrange("c r w -> c (r w)"))
                hn = ph3.tile([C, 2 * W], F32, tag="hn")
                nc.vector.tensor_scalar(hn[:], hsb[:], s_rstd[:], s_nbias[:],
                                        ALU.mult, ALU.add)
                ps = psum_conv.tile([C, 2 * W], F32, tag="conv")
                nc.tensor.matmul(ps[:], s_p1w[:], hn[:], start=True, stop=True)
                tt_ = ph3.tile([C, 2 * W], F32, tag="tt")
                nc.scalar.activation(tt_[:], ps[:], ACTF.Gelu, bias=s_p1b[:])
                mk = ph3.tile([C, 2], F32, tag="mk")
                nc.sync.dma_start(mk[:], hmask[g, :, :])
                tm = ph3.tile([C, 2 * W], BF16, tag="tm")
                nc.vector.tensor_tensor(
                    tm[:].rearrange("c (r w) -> c r w", r=2),
                    tt_[:].rearrange("c (r w) -> c r w", r=2),
                    mk[:].unsqueeze(2).broadcast_to([C, 2, W]), ALU.mult)
                nc.scalar.dma_start(t_nchw[0:C, 2 * g:2 * g + 2, 2:2 + W],
                                    tm[:].rearrange("c (r w) -> c r w", r=2))
                # dup-shifted copy (cols -1) for paired-tap offset convs
                nc.scalar.dma_start(t_nchw[C:2 * C, 2 * g:2 * g + 2, 1:1 + W],
                                    tm[:].rearrange("c (r w) -> c r w", r=2))
                # NHWC transposed fp8 copies (2x2 corner-dup quad write)
                for bb in range(4):
                    pst_full = psum_tr.tile([128, 128], BF16, tag="trb")
                    pst = pst_full[:, :C]
                    nc.tensor.matmul(pst[:], tm[:, 128 * bb:128 * (bb + 1)],
                                     s_id16b[:C, :C], start=True, stop=True,
                                     is_transpose=True)
                    tT = ph3.tile([128, C], F8, tag="tT")
                    nc.vector.tensor_copy(tT[:], pst[:])
                    l = 2 * g + HL0 + bb // 2
                    xh = bb % 2
                    # partition p holds pixel x = PADC+128*xh+p; its value
                    # lands in 4 packets: (l,x)[0:C], (l,x-1)[C:2C],
                    # (l-1,x)[2C:3C], (l-1,x-1)[3C:4C]. Two dual writes.
                    e0 = ((l + PADR) * NPC + PADC + 128 * xh) * 4 * C
                    src2 = tT[:].unsqueeze(1).broadcast_to([128, 2, C])
                    nc.scalar.dma_start(
                        _ap_raw(t_pad, e0 - 3 * C,
                                [[4 * C, 128], [3 * C, 2], [1, C]]), src2)
                    nc.scalar.dma_start(
                        _ap_raw(t_pad, e0 - 4 * C * NPC - C,
                                [[4 * C, 128], [3 * C, 2], [1, C]]), src2)

        # ---------------- deform stages ----------------
        def deform_stage(name, chunks, KK, d_by, d_bx, d_offw, d_offb, d_dwk,
                         src_nchw, src_pad, scr, win_margin, kspan, out_stage,
                         gp_bufs=2, dp_bufs=2):
            """Emit one deformable depthwise conv stage.

            out_stage(l0, rp, xh, acc_psum, dpool) consumes the [C,128] tap-sum."""
            Kg2 = 2 * KK
            if KK == K1:
                conv_pairs = [(ky, kx) for ky in (-2, -1, 0, 1, 2)
                              for kx in (-2, 0, 2)]
            else:
                conv_pairs = [(3 * ky, 3 * kx) for ky in (-3, -2, -1, 0, 1, 2, 3)
                              for kx in (-3, -1, 1, 3)]
            NPAIR = len(conv_pairs)
            with tc.tile_pool(name=name + "s", bufs=1) as st, \
                 tc.tile_pool(name=name, bufs=dp_bufs) as dp, \
                 tc.tile_pool(name=name + "r", bufs=2) as rp_pool, \
                 tc.tile_pool(name=name + "g", bufs=gp_bufs) as gp:
                s_offw = st.tile([2 * C, NPAIR, Kg2], BF16)
                s_offb = st.tile([Kg2, 1], F32)
                s_dwk = st.tile([128, KK, C], F16)
                s_bw = st.tile([128, 4 * KK], F32)
                s_bxw = st.tile([128, 4 * KK], F32)
                for dst, src in [(s_offw, d_offw), (s_offb, d_offb),
                                 (s_dwk, d_dwk), (s_bw, d_by), (s_bxw, d_bx)]:
                    nc.sync.dma_start(dst[:], src[:])
                gidx = 0
                for (c0, c1) in chunks:
                    win_l0 = c0 - win_margin
                    win_rows = (c1 - c0) + 2 * win_margin
                    n_elems = win_rows * NPC
                    for l0 in range(c0, c1, 2):
                        # ---- offset conv on rows (l0, l0+1) ----
                        if name == "d1":
                            rt = rp_pool.tile([2 * C, 6, W + 4], BF16, tag="rt")
                            nc.sync.dma_start(
                                rt[:], src_nchw[:, (l0 - 2) - HL0:(l0 + 4) - HL0, :])
                            ps = psum_conv.tile([Kg2, 2 * W], F32, tag="conv")
                            for j, (ky, kx) in enumerate(conv_pairs):
                                nc.tensor.matmul(
                                    ps[:].rearrange("c (r w) -> c r w", r=2),
                                    s_offw[:, j, :],
                                    rt[:, (ky + 2):(ky + 4), (kx + 2):(kx + 2) + W],
                                    start=(j == 0), stop=(j == NPAIR - 1))
                        else:
                            rt = rp_pool.tile([2 * C, 20, W + 18], BF16, tag="rt")
                            nc.sync.dma_start(
                                rt[:], src_nchw[:, (l0 - 9) - A1L0:(l0 + 11) - A1L0, :])
                            ps = psum_conv.tile([Kg2, 2 * W], F32, tag="conv")
                            for j, (ky, kx) in enumerate(conv_pairs):
                                nc.tensor.matmul(
                                    ps[:].rearrange("c (r w) -> c r w", r=2),
                                    s_offw[:, j, :],
                                    rt[:, (ky + 9):(ky + 11), (kx + 9):(kx + 9) + W],
                                    start=(j == 0), stop=(j == NPAIR - 1))
                        osb = dp.tile([Kg2, 2 * W], BF16, tag="osb")
                        nc.scalar.activation(osb[:], ps[:], ACTF.Identity, bias=s_offb[:])
                        # ---- transpose offsets to [px, ch] for 4 batches ----
                        offsT = dp.tile([128, 4 * Kg2], F32, tag="offsT")
                        for bb in range(4):
                            pst_full = psum_tr.tile([128, 128], BF16, tag="trb")
                            pst = pst_full[:, :Kg2]
                            nc.tensor.matmul(pst[:], osb[:, 128 * bb:128 * (bb + 1)],
                                             s_id16b[:Kg2, :Kg2], start=True, stop=True,
                                             is_transpose=True)
                            nc.vector.tensor_copy(
                                offsT[:, Kg2 * bb:Kg2 * (bb + 1)], pst[:])
                        # ---- index & weight prep on [128, 4*KK] views ----
                        yv = offsT[:].rearrange("p (b c) -> p b c", b=4)[:, :, 0:KK]
                        xv = offsT[:].rearrange("p (b c) -> p b c", b=4)[:, :, KK:Kg2]
                        py = dp.tile([128, 4 * KK], F32, tag="py")
                        px = dp.tile([128, 4 * KK], F32, tag="px")
                        pyv = py[:].rearrange("p (b k) -> p b k", b=4)
                        pxv = px[:].rearrange("p (b k) -> p b k", b=4)
                        nc.vector.tensor_tensor(pyv, yv, s_bw[:].rearrange(
                            "p (b k) -> p b k", b=4), ALU.add)
                        nc.vector.tensor_scalar(py[:], py[:], float(l0 - win_l0),
                                                None, ALU.add)
                        nc.vector.tensor_tensor(pxv, xv, s_bxw[:].rearrange(
                            "p (b k) -> p b k", b=4), ALU.add)
                        # y0 = round(py - 0.5) via the fp32 magic-number trick;
                        # equals floor(py) except exact-integer ties, which
                        # still yield a valid (y0, fy=py-y0) bilinear pair.
                        MAGIC = 8388608.0
                        y0 = dp.tile([128, 4 * KK], F32, tag="y0")
                        x0 = dp.tile([128, 4 * KK], F32, tag="x0")
                        nc.vector.tensor_scalar(y0[:], py[:], MAGIC - 0.5,
                                                -MAGIC, ALU.add, ALU.add)
                        nc.vector.tensor_scalar(x0[:], px[:], MAGIC - 0.5,
                                                -MAGIC, ALU.add, ALU.add)
                        fy = dp.tile([128, 4 * KK], F32, tag="fy")
                        fx = dp.tile([128, 4 * KK], F32, tag="fx")
                        nc.vector.tensor_tensor(fy[:], py[:], y0[:], ALU.subtract)
                        nc.vector.tensor_tensor(fx[:], px[:], x0[:], ALU.subtract)
                        nc.vector.tensor_scalar(y0[:], y0[:], float(win_rows - 2),
                                                0.0, ALU.min, ALU.max)
                        nc.vector.tensor_scalar(x0[:], x0[:], float(NPC - 2),
                                                0.0, ALU.min, ALU.max)
                        idxf = dp.tile([128, 4 * KK], F32, tag="idxf")
                        nc.vector.scalar_tensor_tensor(idxf[:], y0[:], float(NPC),
                                                       x0[:], ALU.mult, ALU.add)
                        idxa = dp.tile([128, 4 * KK], I16, tag="idxa")
                        nc.vector.tensor_copy(idxa[:], idxf[:])
                        fyb = dp.tile([128, 4 * KK], F32, tag="fyb")
                        fxb = dp.tile([128, 4 * KK], F32, tag="fxb")
                        nc.vector.tensor_scalar(fyb[:], fy[:], -1.0, 1.0,
                                                ALU.mult, ALU.add)
                        nc.vector.tensor_scalar(fxb[:], fx[:], -1.0, 1.0,
                                                ALU.mult, ALU.add)
                        # bilinear weights interleaved (k, ij) for the batched
                        # product: w4[p, 4*(b*KK+k) + 2*iy + ix]
                        w4 = dp.tile([128, 4 * KK * 4], F16, tag="w4")
                        w4v = w4[:].rearrange("p (k f) -> p k f", f=4)
                        nc.vector.tensor_tensor(w4v[:, :, 0], fyb[:], fxb[:],
                                                ALU.mult)
                        nc.vector.tensor_tensor(w4v[:, :, 1], fyb[:], fx[:],
                                                ALU.mult)
                        nc.vector.tensor_tensor(w4v[:, :, 2], fy[:], fxb[:],
                                                ALU.mult)
                        nc.vector.tensor_tensor(w4v[:, :, 3], fy[:], fx[:],
                                                ALU.mult)
                        # ---- rewrap indices via DRAM bounce (one read DMA,
                        # replicating the 16-partition wrap into all 8 groups)
                        sb = scr[gidx % NSCR]
                        nc.sync.dma_start(sb[:, :], idxa[:])
                        wrapped = gp.tile([128, 4 * KK * 8], I16, tag="wrp")
                        for rep in range(8):
                            nc.sync.dma_start(
                                wrapped[16 * rep:16 * (rep + 1), :].rearrange(
                                    "r (b g q) -> r b g q", b=4, g=KK),
                                sb[:].rearrange("(q r) (b g) -> r b g q",
                                                q=8, b=4))
                        gidx += 1
                        # ---- per batch: gather fp8 2x2 packets + combine ----
                        for bb in range(4):
                            rp, xh = bb // 2, bb % 2
                            win_off = (win_l0 + PADR) * NPC * 4 * C
                            gsets = ([(0, KK)] if KK == K1
                                     else [(0, 25), (25, KK)])
                            gtiles = []
                            for gi, (ka, kb) in enumerate(gsets):
                                ng = kb - ka
                                gt = gp.tile([128, 25, 4 * C], F8, tag="gth")
                                inap = _ap_raw(
                                    src_pad, win_off,
                                    [[4 * C, n_elems], [1, 4 * C]])
                                nc.gpsimd.dma_gather(
                                    gt[:, :ng, :],
                                    inap,
                                    wrapped[:, bb * KK * 8 + ka * 8:
                                            bb * KK * 8 + kb * 8],
                                    ng * 128, gather_regs[ng * 128], 4 * C,
                                    4 * C, single_packet=False,
                                    queue_num=(bb * len(gsets) + gi) % 4)
                                gtiles.append((ka, kb, gt))
                            acc = psum_acc.tile([128, 128], F16, tag="acc")
                            for (ka, kb, gt) in gtiles:
                                nk = kb - ka
                                # batched bilinear: one product pass over the
                                # whole gather tile (broadcast weights), then
                                # 2x pair-sums and one 2x dwk multiply.
                                gv = gt[:, 0:nk, :].rearrange(
                                    "p a b -> p (a b)").rearrange(
                                    "p (k f c) -> p k f c", f=4, c=C)
                                wv = w4[:, 4 * (bb * KK + ka):
                                        4 * (bb * KK + kb)].rearrange(
                                    "p (k f) -> p k f", f=4).unsqueeze(
                                    3).broadcast_to([128, nk, 4, C])
                                m = dp.tile([128, 25, 4 * C], F16, tag="m")
                                mv = m[:, 0:nk, :].rearrange(
                                    "p a b -> p (a b)").rearrange(
                                    "p (k f c) -> p k f c", f=4, c=C)
                                nc.vector.tensor_tensor(mv, gv, wv, ALU.mult)
                                # x-pair sum: [p, (k,iy), 2C] halves
                                mr = m[:, 0:nk, :].rearrange(
                                    "p k (i c2) -> p (k i) c2", i=2)
                                s1 = dp.tile([128, 2 * 25, C], F16, tag="s1")
                                nc.vector.tensor_tensor(
                                    s1[:, 0:2 * nk, :], mr[:, :, 0:C],
                                    mr[:, :, C:2 * C], ALU.add)
                                s1v = s1[:, 0:2 * nk, :].rearrange(
                                    "p (k i) c -> p k i c", i=2)
                                sw = dp.tile([128, 25, C], F16, tag="sw")
                                nc.vector.tensor_tensor(
                                    sw[:, 0:nk], s1v[:, :, 0, :],
                                    s1v[:, :, 1, :], ALU.add)
                                nc.vector.tensor_tensor(
                                    sw[:, 0:nk], sw[:, 0:nk],
                                    s_dwk[:, ka:kb, :], ALU.mult)
                                # pair two taps per transpose: out partitions
                                # [0:C] = tap j, [C:2C] = tap j+1
                                j = ka
                                while j < kb:
                                    take = min(2, kb - j)
                                    lhs = sw[:, j - ka:j - ka + take, :].rearrange(
                                        "p k c -> p (k c)")
                                    nc.tensor.matmul(
                                        acc[:take * C, :], lhs, s_id16[:],
                                        start=(j == 0), stop=(j + take == KK),
                                        is_transpose=True)
                                    j += take
                            asumh = dp.tile([C, 128], BF16, tag="asumh")
                            nc.scalar.activation(asumh[:], acc[0:C, :], ACTF.Copy)
                            asum = dp.tile([C, 128], BF16, tag="asum")
                            nc.vector.tensor_tensor(asum[:], asumh[:],
                                                    acc[C:2 * C, :], ALU.add)
                            out_stage(l0, rp, xh, asum, dp)

        # ---------------- deform1 consumer: write a1 ----------------
        def a1_out(l0, rp, xh, a1sb, dpool):
            l = l0 + rp
            nc.scalar.dma_start(
                a1_nchw[0:C, l - A1L0,
                        9 + 128 * xh: 9 + 128 * (xh + 1)], a1sb[:])
            # dup-shifted copy (cols -3) for paired-tap offset convs
            nc.scalar.dma_start(
                a1_nchw[C:2 * C, l - A1L0,
                        6 + 128 * xh: 6 + 128 * (xh + 1)], a1sb[:])
            pst_full = psum_tr.tile([128, 128], BF16, tag="trb")
            pst = pst_full[:, :C]
            nc.tensor.matmul(pst[:], a1sb[:], s_id16b[:C, :C], start=True,
                             stop=True, is_transpose=True)
            a1T = dpool.tile([128, C], F8, tag="a1T")
            nc.vector.tensor_copy(a1T[:], pst[:])
            e0 = ((l + PADR) * NPC + PADC + 128 * xh) * 4 * C
            src2 = a1T[:].unsqueeze(1).broadcast_to([128, 2, C])
            nc.scalar.dma_start(
                _ap_raw(a1_pad, e0 - 3 * C,
                        [[4 * C, 128], [3 * C, 2], [1, C]]), src2)
            nc.scalar.dma_start(
                _ap_raw(a1_pad, e0 - 4 * C * NPC - C,
                        [[4 * C, 128], [3 * C, 2], [1, C]]), src2)

        def dump_to_out(src_dram, row_off, col_off, row_len):
            dt = src_dram[:].dtype
            with tc.tile_pool(name="dump", bufs=2) as dmp:
                for g in range(64):
                    src = src_dram[0:C, row_off + 2 * g:row_off + 2 * g + 2,
                                   col_off:col_off + W]
                    if dt == F32R:
                        src = src.bitcast(F32)
                        dt = F32
                    tl = dmp.tile([C, 2, W], dt, tag="dt")
                    nc.sync.dma_start(tl[:], src)
                    if dt != F32:
                        tf = dmp.tile([C, 2, W], F32, tag="dtf")
                        nc.vector.tensor_copy(tf[:], tl[:])
                        tl = tf
                    nc.scalar.dma_start(out_t[:, 2 * g:2 * g + 2, :], tl[:])

        if PHASES >= 4:
            deform_stage("d1", D1_CHUNKS, K1, by1, bx1, off0w, off0b,
                         dwk1, t_nchw, t_pad, scr1, WIN1, 2, a1_out)

        # ---------------- deform2 consumer: tail fusion ----------------
        def tail_out(l0, rp, xh, a2sb, dpool):
            l = l0 + rp
            psg = psum_tail.tile([C, 128], F32, tag="tail")
            nc.tensor.matmul(psg[:], s_g1w[:], a2sb[:], start=True, stop=True)
            ut = dpool.tile([C, 128], BF16, tag="ut")
            nc.sync.dma_start(
                ut[:], t_nchw[0:C, l - HL0,
                              2 + 128 * xh: 2 + 128 * (xh + 1)])
            t2 = dpool.tile([C, 128], BF16, tag="t2")
            nc.vector.scalar_tensor_tensor(t2[:], psg[:], s_g1b[:], ut[:],
                                           ALU.add, ALU.mult)
            psp = psum_tail.tile([C, 128], F32, tag="tail")
            nc.tensor.matmul(psp[:], s_p2w[:], t2[:], start=True, stop=True)
            ht = dpool.tile([C, 128], F32, tag="ht")
            nc.sync.dma_start(
                ht[:], h_local[:, l - HL0, 128 * xh: 128 * (xh + 1)])
            v1 = dpool.tile([C, 128], F32, tag="v1")
            nc.scalar.activation(v1[:], psp[:], ACTF.Identity, bias=s_cb[:])
            v2 = dpool.tile([C, 128], F32, tag="v2")
            nc.vector.scalar_tensor_tensor(v2[:], ht[:], s_rstd[:], v1[:],
                                           ALU.mult, ALU.add)
            v3 = dpool.tile([C, 128], F32, tag="v3")
            nc.vector.scalar_tensor_tensor(v3[:], v2[:], 0.2, v2[:],
                                           ALU.mult, ALU.max)
            nc.scalar.dma_start(out_t[:, l, 128 * xh: 128 * (xh + 1)], v3[:])

        if PHASES >= 5:
            deform_stage("d2", D2_CHUNKS, K2, by2, bx2, offsw, offsb,
                         dwk2, a1_nchw, a1_pad, scr2, WIN2, 6, tail_out)
        elif PHASES == 2:
            dump_to_out(h_local, -HL0, 0, 128)
        elif PHASES == 3:
            dump_to_out(t_nchw, -HL0, 2, 128)
        elif PHASES == 4:
            dump_to_out(a1_nchw, -A1L0, 9, 128)
        elif PHASES <= 1:
            dump_to_out(h_local, -HL0, 0, 128)

    nc.compile()
    return nc


def prepare_inputs(inputs):
    """Host-side marshaling: returns in_maps (list of 8 dicts)."""
    import ml_dtypes
    bf16 = ml_dtypes.bfloat16
    x = inputs["x"].astype(np.float32)
    conv_w = inputs["conv_w"].astype(np.float32)

    def reorder(idx_list):
        return np.array(idx_list, dtype=np.int64)

    def pack_pairs(wfull, ksz, kxs):
        """wfull [cin, ky, kx, cout] -> [2C, npair, cout]; lower half = base
        tap, upper = (kx+1) partner (zero when absent)."""
        cout = wfull.shape[3]
        npair = ksz * len(kxs)
        out = np.zeros((2 * C, npair, cout), np.float32)
        for j, (ky, kx) in enumerate([(a, b) for a in range(ksz) for b in kxs]):
            out[:C, j] = wfull[:, ky, kx]
            if kx + 1 < ksz:
                out[C:, j] = wfull[:, ky, kx + 1]
        return out

    # conv3x3 paired lhsT: [2C, 6, cout]
    cw3 = pack_pairs(conv_w.transpose(1, 2, 3, 0), 3, (0, 2))
    p1w = np.ascontiguousarray(inputs["p1_w"].T)
    p1b = inputs["p1_b"].reshape(C, 1).astype(np.float32)
    # offset convs: reorder output channels to [y-taps, x-taps]
    ord1 = np.concatenate([np.arange(0, 2 * K1, 2), np.arange(1, 2 * K1, 2)])
    off0w = pack_pairs(inputs["off0_w"][ord1].transpose(1, 2, 3, 0),
                       5, (0, 2, 4)).astype(bf16)
    off0b = inputs["off0_b"][ord1].reshape(2 * K1, 1).astype(np.float32)
    ord2 = np.concatenate([np.arange(0, 2 * K2, 2), np.arange(1, 2 * K2, 2)])
    offsw = pack_pairs(inputs["offs_w"][ord2].transpose(1, 2, 3, 0),
                       7, (0, 2, 4, 6)).astype(bf16)
    offsb = inputs["offs_b"][ord2].reshape(2 * K2, 1).astype(np.float32)
    dwk1 = np.broadcast_to(
        inputs["dw0_w"].reshape(C, K1).T[None, :, :], (128, K1, C)).astype(np.float16)
    dwk2 = np.broadcast_to(
        inputs["dws_w"].reshape(C, K2).T[None, :, :], (128, K2, C)).astype(np.float16)
    g1w = np.ascontiguousarray(inputs["g1_w"].T).astype(bf16)
    g1b = inputs["g1_b"].reshape(C, 1).astype(np.float32)
    p2w = np.ascontiguousarray(inputs["p2_w"].T).astype(bf16)
    p2b = inputs["p2_b"].reshape(C, 1).astype(np.float32)
    ident = np.eye(128, dtype=np.float32)
    ident16 = np.eye(128, dtype=np.float16)
    ident16b = np.eye(128, dtype=bf16)

    def btiles(KK, dil, ksz, win_margin):
        ky = dil * (np.arange(KK) // ksz - (ksz // 2))
        kx = dil * (np.arange(KK) % ksz - (ksz // 2))
        p = np.arange(128)
        by = np.zeros((128, 4 * KK), np.float32)
        bx = np.zeros((128, 4 * KK), np.float32)
        for bb in range(4):
            rp, xh = bb // 2, bb % 2
            by[:, bb * KK:(bb + 1) * KK] = ky[None, :] + win_margin + rp
            bx[:, bb * KK:(bb + 1) * KK] = (kx[None, :] + p[:, None]
                                            + PADC + 128 * xh)
        return by, bx

    by1_, bx1_ = btiles(K1, 1, 5, WIN1)
    by2_, bx2_ = btiles(K2, 3, 7, WIN2)

    phases = int(os.environ.get("KERNEL_PHASES", "5"))
    common = dict(cw3=cw3, p1w=p1w, p1b=p1b, off0w=off0w, off0b=off0b,
                  offsw=offsw, offsb=offsb, dwk1=dwk1, dwk2=dwk2,
                  g1w=g1w, g1b=g1b, p2w=p2w, p2b=p2b, identw=ident,
                  identw16=ident16, identw16b=ident16b,
                  phtag=np.zeros((1, phases + 1), np.float32),
                  by1=by1_, bx1=bx1_, by2=by2_, bx2=bx2_)

    in_maps = []
    for core in range(N_CORES):
        b, half = core // 2, core % 2
        r0 = 128 * half
        xi = x[b]  # [C,H,W]
        # x_extra: the OTHER half's rows +-1, for instance-norm stats
        o0 = 128 - r0  # other half start
        x_extra = np.zeros((2 * C, 130, W + 2), np.float32)
        lo, hi = o0 - 1, o0 + 129
        vlo, vhi = max(lo, 0), min(hi, H)
        x_extra[:C, vlo - lo:vhi - lo, 1:-1] = xi[:, vlo:vhi, :]
        x_extra[C:, :, 0:W + 1] = x_extra[:C, :, 1:W + 2]
        # x_local rows: img rows r0-25 .. r0+152 (178 rows), cols pad 1
        x_local = np.zeros((2 * C, NHROWS + 2, W + 2), np.float32)
        lo, hi = r0 - 25, r0 + 153
        vlo, vhi = max(lo, 0), min(hi, H)
        x_local[:C, vlo - lo:vhi - lo, 1:-1] = xi[:, vlo:vhi, :]
        x_local[C:, :, 0:W + 1] = x_local[:C, :, 1:W + 2]
        # hmask: group g covers local rows (2g-24, 2g-23); per-row validity
        hmask = np.zeros((NHROWS // 2, C, 2), np.float32)
        for g in range(NHROWS // 2):
            for rr in range(2):
                l = 2 * g + HL0 + rr
                if 0 <= r0 + l < H:
                    hmask[g, :, rr] = 1.0
        m = dict(common)
        m.update(x_extra=x_extra, x_local=x_local, hmask=hmask)
        in_maps.append(m)
    return in_maps


_CACHED = {}


LAST_EXEC_NS = None


def kernel(**inputs):
    global LAST_EXEC_NS
    if "nc" not in _CACHED:
        _CACHED["nc"] = build_program()
    nc = _CACHED["nc"]
    in_maps = prepare_inputs(inputs)
    trace = bool(int(os.environ.get("KERNEL_TRACE", "0")))
    res = run_bass_kernel_spmd(nc, in_maps, list(range(N_CORES)), trace=trace)
    if res.exec_time_ns is not None:
        LAST_EXEC_NS = res.exec_time_ns
    out = np.zeros((B, C, H, W), np.float32)
    for core in range(N_CORES):
        b, half = core // 2, core % 2
        out[b, :, 128 * half:128 * (half + 1), :] = res.results[core]["out"]
    return out


if __name__ == "__main__":
    import reference as R
    inp = {k: np.asarray(v) for k, v in R.setup_inputs().items()}
    o = kernel(**inp)
    ref = np.load("/root/problem/ref_out.npy")
    err = np.abs(o - ref).max() / (np.abs(ref).max() + 1e-9)
    print("rel err:", err)

